# revision 29
# baseline (speedup 1.0000x reference)
"""Trainium2 Bass kernel for CareGptOssAttentionHF (MLA-style sliding-window
attention with sinks).

Sharding: sequence-parallel across 8 NeuronCores. Core c owns query rows
[c*256, (c+1)*256) and redundantly computes latent/K/V for its 768-row key
halo [c*256-512, c*256+256) — no collectives needed (window = 512).

On-chip dataflow (per core, all fp16 operands, fp32 PSUM accumulation):
  qT   [d=128, h, i]   = WqT.T @ hsT_local (+bq): one DMA-paced k-pass with 16
                         PSUM accumulators (all 8 banks); RoPE on rows 64:128
  latT [r=576, j]      = WkvaT.T @ hsT_halo (+bkva), then zeroed for halo rows
                         with j<0 (multiplicative jmask) so padded keys vanish
  kfT  [d=128, h, j]   : rows 0:64 = w_kc-projected nope, rows 64:128 = RoPE'd
                         shared k_rope broadcast per head (copies split ACT/DVE)
  v65  [j, h*65+d]     = latT.T @ wvc (key-major; col 64 of each head = jvalid
                         {0,1}, so the q-major PV matmul's column 64 yields the
                         softmax denominator restricted to real keys)
  scoresT[j, i] per h  -> one matmul per key block at N=256 covering both query
                         tiles; the 2 universally-masked (it,jb) pairs are
                         skipped -> exp(SCALE*x) on ACT -> {0,1} mask only on
                         the 4 partial diagonal blocks (strided pair APs)
  PV (q-major)         : out[i, v|den] per (h, it); reciprocal on a [128,1]
                         column; normalize via DVE per-partition scalar; oat
                         transposed back to hv-major with 16 PE transposes
  out = oatT.T @ WoT (+bo), fp32
"""

import os
import sys

import numpy as np

if "/opt/trn_rl_repo" not in sys.path:
    sys.path.insert(0, "/opt/trn_rl_repo")

B, S, HID, H = 1, 2048, 2048, 16
NOPE = ROPE = 64
D = NOPE + ROPE  # 128
V = 64
R = 512
SW = 512
NCORES = 8
Q = S // NCORES  # 256 query rows per core
KH = Q + SW  # 768 halo key rows per core
NJB = KH // 128  # 6 key blocks
NIT = Q // 128  # 2 query tiles
SCALE = float(D) ** -0.5
DEBUG = bool(int(os.environ.get("BASSDBG", "0")))

# valid jb sets per query tile: (it0, jb5) has j>i always, (it1, jb0) has
# i-j>=512 always -> both skipped on every core
JB_IT = (tuple(range(0, 5)), tuple(range(1, 6)))
# (it, jb) pairs needing a partial causal/window mask, in mask_sb slot order
MASK_SLOTS = ((0, 0), (0, 4), (1, 1), (1, 5))

_CACHE = {}


def _build_program():
    import concourse.bass as bass
    import concourse.mybir as mybir
    from concourse import tile
    from contextlib import ExitStack

    f32 = mybir.dt.float32
    f16 = mybir.dt.float16
    AF = mybir.ActivationFunctionType
    OP = mybir.AluOpType

    nc = bass.Bass()

    # all matrices pre-packed on host into SBUF layout ([partition, free]
    # with large contiguous rows) so each load is 128 big DMA descriptors
    # at full HBM rate instead of thousands of ~1KB ones
    hsT_d = nc.dram_tensor("hsT", [128, 16 * KH], f16, kind="ExternalInput")
    wqT_d = nc.dram_tensor("wqT", [128, 16 * H * D], f16, kind="ExternalInput")
    wkvaT_d = nc.dram_tensor("wkvaT", [128, 16 * (R + ROPE)], f16, kind="ExternalInput")
    wkc_d = nc.dram_tensor("wkc", [128, 4 * H * NOPE], f16, kind="ExternalInput")
    wvc_d = nc.dram_tensor("wvc", [128, 4 * H * V], f16, kind="ExternalInput")
    woT_d = nc.dram_tensor("woT", [128, 8 * HID], f16, kind="ExternalInput")
    bq_d = nc.dram_tensor("bq", [128, 16], f32, kind="ExternalInput")
    bkva_d = nc.dram_tensor("bkva", [128, 5], f32, kind="ExternalInput")
    bo_d = nc.dram_tensor("bo", [128, HID], f16, kind="ExternalInput")
    tqc_d = nc.dram_tensor("trigq_cos", [128, Q], f16, kind="ExternalInput")
    tqs_d = nc.dram_tensor("trigq_sin", [128, Q], f16, kind="ExternalInput")
    tk_d = nc.dram_tensor("trigk", [64, 2, KH], f16, kind="ExternalInput")
    mask_d = nc.dram_tensor("mask4", [128, 4, 128], f16, kind="ExternalInput")
    jmask_d = nc.dram_tensor("jmask", [128, KH], f16, kind="ExternalInput")
    jvalid_d = nc.dram_tensor("jvalid", [128, NJB], f16, kind="ExternalInput")
    ident_d = nc.dram_tensor("ident", [128, 128], f16, kind="ExternalInput")
    esink_d = nc.dram_tensor("esink", [128, H], f32, kind="ExternalInput")
    out_d = nc.dram_tensor("out", [Q, HID], f32, kind="ExternalOutput")

    dbg = {}
    if DEBUG:
        dbg["lat"] = nc.dram_tensor("dbg_lat", [128, 4, KH], f16, kind="ExternalOutput")
        dbg["lat4"] = nc.dram_tensor("dbg_lat4", [64, KH], f16, kind="ExternalOutput")
        dbg["q"] = nc.dram_tensor("dbg_q", [128, H, Q], f16, kind="ExternalOutput")
        dbg["kf"] = nc.dram_tensor("dbg_kf", [128, H, KH], f16, kind="ExternalOutput")
        dbg["v"] = nc.dram_tensor("dbg_v", [128, NJB, H * 65], f16, kind="ExternalOutput")
        dbg["pr"] = nc.dram_tensor("dbg_pr", [128, H, NJB, Q], f16, kind="ExternalOutput")
        dbg["oatq"] = nc.dram_tensor("dbg_oatq", [128, NIT, H * V], f16, kind="ExternalOutput")
        dbg["oat"] = nc.dram_tensor("dbg_oat", [128, 8, Q], f16, kind="ExternalOutput")

    with tile.TileContext(nc) as tc, ExitStack() as ctx:
        const = ctx.enter_context(tc.tile_pool(name="const", bufs=1))

        # ---- resident tiles (DMAs issued below in ring order) ----
        hs = const.tile([128, 16, KH], f16)
        wkva = const.tile([128, 16, R + ROPE], f16)
        wkc = const.tile([128, 4, H * NOPE], f16)
        wvc = const.tile([128, 4, H * V], f16)
        bq_sb = const.tile([128, 16], f32)
        bkva_sb = const.tile([128, 5], f32)
        bo_sb = const.tile([128, HID], f16)
        tqc = const.tile([128, Q], f16)
        tqs = const.tile([128, Q], f16)
        tk = const.tile([64, 2, KH], f16)
        mask_sb = const.tile([128, 4, 128], f16)
        jmask_sb = const.tile([128, KH], f16)
        jvalid_sb = const.tile([128, NJB], f16)
        ident_sb = const.tile([128, 128], f16)
        esink_sb = const.tile([128, H], f32)
        wo_sb = const.tile([128, 8, HID], f16)

        qT = const.tile([128, H, Q], f16)
        latbf = const.tile([128, 4, KH], f16)
        lat4 = const.tile([64, KH], f16)
        oatq = const.tile([128, NIT, H * V], f16)
        out_sb = const.tile([128, NIT, HID], f32)
        # rotation scratch lives in the const pool, NOT the Wq-reuse region:
        # a reused-region tile's first write would wait for all of phase 1's
        # Wq reads, serializing RoPE behind the q projection
        rotq = const.tile([128, 8, Q], f16)
        rotk = const.tile([64, KH], f16)

        def bc(ap, n):
            # broadcast a [P, F] AP to [P, n, F] via a step-0 middle dim
            return bass.AP(ap.tensor, ap.offset, [ap.ap[0], [0, n], ap.ap[1]])

        # ---- input DMAs + phase 3 + phase 1 ----
        # Ring order = arrival order: hs and wkva land first so the latent
        # projection (phase 3) starts ~15us in and covers the Wq transfer;
        # phase 1 then runs at full PE speed with Wq fully resident. Wq is
        # loaded whole into an 8MB region with first-use (zero-wait) DMAs;
        # the region is released afterward and reused by kf/v65/etc, whose
        # writers are compute engines (multi-wait capable), never DMAs —
        # DMA ring entries can carry at most ONE embedded wait.
        with tc.tile_pool(name="wqfull", bufs=1) as wqp:
            wq = wqp.tile([128, 16, H * D], f16)
            nc.sync.dma_start(wkva[:], wkvaT_d[:])
            for c in range(4):
                nc.sync.dma_start(
                    hs[:, 4 * c : 4 * c + 4, :],
                    hsT_d[:, 4 * c * KH : (4 * c + 4) * KH],
                )
            for c in range(2):
                nc.sync.dma_start(
                    wq[:, 8 * c : 8 * c + 8, :],
                    wqT_d[:, 8 * c * H * D : (8 * c + 8) * H * D],
                )
            nc.sync.dma_start(wkc[:], wkc_d[:])
            nc.sync.dma_start(wvc[:], wvc_d[:])
            # loaded last: first consumed by the final output projection
            nc.sync.dma_start(wo_sb[:], woT_d[:])
            # small constants ride the otherwise-idle ACT HWDGE queue so they
            # land in ~4us and never gate the latbf/RoPE epilogues
            nc.scalar.dma_start(bq_sb[:], bq_d[:])
            nc.scalar.dma_start(bkva_sb[:], bkva_d[:])
            nc.scalar.dma_start(tqc[:], tqc_d[:])
            nc.scalar.dma_start(tqs[:], tqs_d[:])
            nc.scalar.dma_start(tk[:], tk_d[:])
            nc.scalar.dma_start(mask_sb[:], mask_d[:])
            nc.scalar.dma_start(jmask_sb[:], jmask_d[:])
            nc.scalar.dma_start(jvalid_sb[:], jvalid_d[:])
            nc.scalar.dma_start(ident_sb[:], ident_d[:])
            nc.scalar.dma_start(esink_sb[:], esink_d[:])
            nc.scalar.dma_start(bo_sb[:], bo_d[:])

            # ---- phase 3: latent projection (+bias, zero j<0 halo rows) ----
            with tc.tile_pool(name="pslat", bufs=1, space="PSUM") as pslatp:
                pslat = [
                    pslatp.tile([128, KH], f32, tag=f"pslat{m}", name=f"pslat{m}")
                    for m in range(4)
                ]
                for k in range(16):
                    for m in range(4):
                        for n0, n1 in ((0, 512), (512, KH)):
                            nc.tensor.matmul(
                                pslat[m][:, n0:n1],
                                lhsT=wkva[:, k, m * 128 : (m + 1) * 128],
                                rhs=hs[:, k, n0:n1],
                                start=(k == 0),
                                stop=(k == 15),
                            )
                for m in range(4):
                    # latbf = (psum + bkva) * jmask : rows with j<0 become
                    # exactly zero so padded halo keys contribute nothing
                    nc.vector.scalar_tensor_tensor(
                        latbf[:, m, :], pslat[m][:], bkva_sb[:, m : m + 1],
                        jmask_sb[:], OP.add, OP.mult,
                    )
                ps4 = pslatp.tile([64, KH], f32, tag="pslat0")
                for k in range(16):
                    for n0, n1 in ((0, 512), (512, KH)):
                        nc.tensor.matmul(
                            ps4[:, n0:n1],
                            lhsT=wkva[:, k, 512:576],
                            rhs=hs[:, k, n0:n1],
                            start=(k == 0),
                            stop=(k == 15),
                        )
                nc.vector.scalar_tensor_tensor(
                    lat4[:], ps4[:], bkva_sb[0:64, 4:5], jmask_sb[0:64, :],
                    OP.add, OP.mult,
                )

            # ---- phase 1 + RoPE-q: q projection (Wq resident by now); each
            # head-group's RoPE runs on DVE during the next group's matmuls
            def rope_q(hb):
                hs_ = slice(hb * 8, hb * 8 + 8)
                nc.vector.tensor_copy(rotq[64:96, :, :], qT[96:128, hs_, :])
                nc.vector.tensor_copy(rotq[96:128, :, :], qT[64:96, hs_, :])
                nc.vector.tensor_mul(qT[64:96, hs_, :], qT[64:96, hs_, :], bc(tqc[64:96, :], 8))
                nc.vector.tensor_mul(rotq[64:96, :, :], rotq[64:96, :, :], bc(tqs[64:96, :], 8))
                nc.vector.tensor_sub(qT[64:96, hs_, :], qT[64:96, hs_, :], rotq[64:96, :, :])
                nc.vector.tensor_mul(qT[96:128, hs_, :], qT[96:128, hs_, :], bc(tqc[96:128, :], 8))
                nc.vector.tensor_mul(rotq[96:128, :, :], rotq[96:128, :, :], bc(tqs[96:128, :], 8))
                nc.vector.tensor_add(qT[96:128, hs_, :], qT[96:128, hs_, :], rotq[96:128, :, :])

            with tc.tile_pool(name="psq", bufs=1, space="PSUM") as psqp:
                for g in range(2):
                    psq = [
                        psqp.tile([128, Q], f32, tag=f"psq{m}", name=f"psq{m}")
                        for m in range(8)
                    ]
                    for k in range(16):
                        for m in range(8):
                            fo = g * 1024 + m * 128
                            nc.tensor.matmul(
                                psq[m][:],
                                lhsT=wq[:, k, fo : fo + 128],
                                rhs=hs[:, k, SW:KH],
                                start=(k == 0),
                                stop=(k == 15),
                            )
                    for m in range(8):
                        gm = g * 8 + m
                        # all bias-add drains on ACT: downstream PSUM pools
                        # anti-depend on these, and the DVE stream is busy
                        # with RoPE for many us after phase 1
                        nc.scalar.activation(
                            qT[:, gm, :], psq[m][:], AF.Identity,
                            bias=bq_sb[:, gm : gm + 1], scale=1.0,
                        )
                    rope_q(g)

        # long-lived tiles reusing the released Wq region; writers are all
        # compute engines (ACT/DVE), never DMAs.
        kvp = ctx.enter_context(tc.tile_pool(name="kv", bufs=1))
        kf = kvp.tile([128, H, KH], f16)
        # per-head column layout [v(64) | jvalid(1)] so the q-major PV
        # matmul's output column 64 is the masked softmax denominator
        v65 = kvp.tile([128, NJB, H * 65], f16)
        oat = kvp.tile([128, 8, Q], f16)

        # ---- phase 4: RoPE on k_rope + broadcast into kf rows 64:128 ----
        nc.vector.tensor_copy(rotk[0:32, :], lat4[32:64, :])
        nc.vector.tensor_copy(rotk[32:64, :], lat4[0:32, :])
        nc.vector.tensor_mul(lat4[0:32, :], lat4[0:32, :], tk[0:32, 0, :])
        nc.vector.tensor_mul(rotk[0:32, :], rotk[0:32, :], tk[0:32, 1, :])
        nc.vector.tensor_sub(lat4[0:32, :], lat4[0:32, :], rotk[0:32, :])
        nc.vector.tensor_mul(lat4[32:64, :], lat4[32:64, :], tk[32:64, 0, :])
        nc.vector.tensor_mul(rotk[32:64, :], rotk[32:64, :], tk[32:64, 1, :])
        nc.vector.tensor_add(lat4[32:64, :], lat4[32:64, :], rotk[32:64, :])
        # 16 narrow SBUF->SBUF broadcast copies, alternating ACT/DVE
        # (Pool is ~6x slower at copies — keep it off the critical path)
        for h in range(H):
            if h % 2 == 0:
                nc.scalar.copy(kf[64:128, h, :], lat4[:])
            else:
                nc.vector.tensor_copy(kf[64:128, h, :], lat4[:])

        # ---- phase 5: k_nope into kf rows 0:64 ----
        with tc.tile_pool(name="pskn", bufs=4, space="PSUM") as psknp:
            for m in range(8):
                ps = psknp.tile([128, KH], f32, tag="pskn")
                for k in range(4):
                    for n0, n1 in ((0, 512), (512, KH)):
                        nc.tensor.matmul(
                            ps[:, n0:n1],
                            lhsT=wkc[:, k, m * 128 : (m + 1) * 128],
                            rhs=latbf[:, k, n0:n1],
                            start=(k == 0),
                            stop=(k == 3),
                        )
                # PSUM drains split ACT/DVE (Pool cannot read PSUM)
                nc.scalar.copy(kf[0:64, 2 * m, :], ps[0:64, :])
                nc.vector.tensor_copy(kf[0:64, 2 * m + 1, :], ps[64:128, :])

        # ---- phase 6: V (key-major, jvalid column per head) ----
        with tc.tile_pool(name="psv", bufs=2, space="PSUM") as psvp:
            for jb in range(NJB):
                ps = psvp.tile([128, H * V], f32, tag="psv")
                for k in range(4):
                    for n0, n1 in ((0, 512), (512, 1024)):
                        nc.tensor.matmul(
                            ps[:, n0:n1],
                            lhsT=latbf[:, k, jb * 128 : (jb + 1) * 128],
                            rhs=wvc[:, k, n0:n1],
                            start=(k == 0),
                            stop=(k == 3),
                        )
                vview = v65[:, jb, :].rearrange("p (h d) -> p h d", d=65)
                ps_view = ps[:].rearrange("p (h d) -> p h d", d=V)
                nc.scalar.copy(vview[:, :, 0:V], ps_view)
                nc.scalar.copy(vview[:, :, V : V + 1], bc(jvalid_sb[:, jb : jb + 1], H))

        # ---- phase 7: attention (software-pipelined over heads) ----
        probs_tiles = {}

        with tc.tile_pool(name="att_sbuf", bufs=2) as attp, tc.tile_pool(
            name="att_psum", bufs=2, space="PSUM"
        ) as attps, tc.tile_pool(name="stat", bufs=4) as statp:

            def emit_scores(h):
                ps_s = attps.tile([128, NJB, 256], f32, tag="ps_s")
                # jb0 is only valid for query tile 0, jb5 only for tile 1;
                # jb1..4 cover both tiles in one N=256 matmul
                nc.tensor.matmul(
                    ps_s[:, 0, 0:128],
                    lhsT=kf[:, h, 0:128],
                    rhs=qT[:, h, 0:128],
                    start=True, stop=True,
                )
                for jb in range(1, 5):
                    nc.tensor.matmul(
                        ps_s[:, jb, :],
                        lhsT=kf[:, h, jb * 128 : (jb + 1) * 128],
                        rhs=qT[:, h, :],
                        start=True, stop=True,
                    )
                nc.tensor.matmul(
                    ps_s[:, 5, 128:256],
                    lhsT=kf[:, h, 640:768],
                    rhs=qT[:, h, 128:256],
                    start=True, stop=True,
                )
                pr = attp.tile([128, NJB, 256], f16, tag="pr", bufs=3)
                # exp over exactly the valid (it, jb) rects via strided APs
                nc.scalar.activation(
                    pr[:, 0:5, 0:128], ps_s[:, 0:5, 0:128], AF.Exp,
                    bias=0.0, scale=SCALE,
                )
                nc.scalar.activation(
                    pr[:, 1:6, 128:256], ps_s[:, 1:6, 128:256], AF.Exp,
                    bias=0.0, scale=SCALE,
                )
                if DEBUG:  # corners stay unwritten; zero them for the dump
                    nc.vector.memset(pr[:, 5, 0:128], 0.0)
                    nc.vector.memset(pr[:, 0, 128:256], 0.0)
                # partial causal/window masks: (it0: jb0,jb4), (it1: jb1,jb5),
                # each pair as one strided DVE op
                nc.vector.tensor_mul(
                    pr[:, 0:5:4, 0:128], pr[:, 0:5:4, 0:128], mask_sb[:, 0:2, :]
                )
                nc.vector.tensor_mul(
                    pr[:, 1:6:4, 128:256], pr[:, 1:6:4, 128:256], mask_sb[:, 2:4, :]
                )
                probs_tiles[h] = pr
                if DEBUG:
                    nc.sync.dma_start(dbg["pr"][:, h, :, :], pr[:])

            def emit_pv(h):
                pr = probs_tiles.pop(h)
                for it in range(NIT):
                    jbs = JB_IT[it]
                    ps_o = attps.tile([128, 65], f32, tag="ps_o")
                    for n, jb in enumerate(jbs):
                        nc.tensor.matmul(
                            ps_o[:],
                            lhsT=pr[:, jb, it * 128 : (it + 1) * 128],
                            rhs=v65[:, jb, h * 65 : (h + 1) * 65],
                            start=(n == 0),
                            stop=(n == len(jbs) - 1),
                        )
                    dsc = statp.tile([128, 1], f32, tag="dsc")
                    nc.vector.tensor_scalar(
                        dsc[:], ps_o[:, 64:65], esink_sb[:, h : h + 1], None, OP.add
                    )
                    rcp = statp.tile([128, 1], f32, tag="rcp")
                    nc.vector.reciprocal(rcp[:], dsc[:])
                    nc.vector.tensor_scalar(
                        oatq[:, it, h * V : (h + 1) * V],
                        ps_o[:, 0:V], rcp[:], None, OP.mult,
                    )

            def emit_transpose(m):
                # oatq [i, hv] -> oat [hv, i] via the DMA XBAR on the idle
                # ACT HWDGE queue: zero PE/DVE cost, runs as soon as heads
                # 2m/2m+1 finish normalizing (multi-waits hoisted by the
                # split pass onto the ACT engine stream)
                for it in range(NIT):
                    nc.scalar.dma_start(
                        oat[:, m, it * 128 : (it + 1) * 128],
                        oatq[:, it, m * 128 : (m + 1) * 128],
                        transpose=True,
                    )

            emit_scores(0)
            emit_scores(1)
            for h in range(2, H):
                emit_scores(h)
                emit_pv(h - 2)
                if h % 2 == 1:
                    emit_transpose((h - 3) // 2)
            emit_pv(H - 2)
            emit_pv(H - 1)
            emit_transpose(7)

        # ---- phase 8: output projection (i-major) + bias + store ----
        with tc.tile_pool(name="psf", bufs=1, space="PSUM") as psfp:
            psf = [
                psfp.tile([128, 512], f32, tag=f"psf{i}", name=f"psf{i}")
                for i in range(8)
            ]
            for k in range(8):
                for it in range(NIT):
                    for n in range(4):
                        nc.tensor.matmul(
                            psf[it * 4 + n][:],
                            lhsT=oat[:, k, it * 128 : (it + 1) * 128],
                            rhs=wo_sb[:, k, n * 512 : (n + 1) * 512],
                            start=(k == 0),
                            stop=(k == 7),
                        )
            for it in range(NIT):
                for n in range(4):
                    nc.vector.tensor_add(
                        out_sb[:, it, n * 512 : (n + 1) * 512],
                        psf[it * 4 + n][:],
                        bo_sb[:, n * 512 : (n + 1) * 512],
                    )
                    # HWDGE store (SWDGE measured ~4x slower) on the
                    # otherwise-empty ACT queue: first ring entries, so each
                    # carries just the DVE producer wait
                    nc.scalar.dma_start(
                        out_d[it * 128 : (it + 1) * 128, n * 512 : (n + 1) * 512],
                        out_sb[:, it, n * 512 : (n + 1) * 512],
                    )

        if DEBUG:
            nc.sync.dma_start(dbg["lat"][:], latbf[:])
            nc.sync.dma_start(dbg["lat4"][:], lat4[:])
            nc.sync.dma_start(dbg["q"][:], qT[:])
            nc.sync.dma_start(dbg["kf"][:], kf[:])
            nc.sync.dma_start(dbg["v"][:], v65[:])
            nc.sync.dma_start(dbg["oatq"][:], oatq[:])
            nc.sync.dma_start(dbg["oat"][:], oat[:])

    if not bool(int(os.environ.get("BASSNOSPLIT", "0"))):
        _split_multi_waits(nc, mybir)
    nc.finalize()
    return nc


def _split_multi_waits(nc, mybir):
    """The TPB ISA has a single embedded wait slot per instruction and this
    toolchain's walrus pass list has no wait-splitting pass ("Too many sync
    wait commands"). Hoist all-but-one wait of every multi-wait compute
    instruction into standalone same-engine EventSemaphore instructions
    placed immediately before it. DMA ring entries can't be split this way
    (they don't execute in the engine stream) — the kernel is structured so
    every DMA already has <=1 wait; assert that here."""
    seq_ok = (mybir.InstEventSemaphore,)
    n = 0
    for fn in nc.m.functions:
        for blk in fn.blocks:
            out = []
            for inst in blk.instructions:
                si = inst.sync_info
                if si is not None and len(si.on_wait) > 1 and not isinstance(inst, seq_ok):
                    if isinstance(inst, mybir.InstDMACopy) and inst.engine not in (
                        mybir.EngineType.SP,
                        mybir.EngineType.Activation,
                    ):
                        # HWDGE DMA_DIRECT2D executes in the SP/ACT engine
                        # stream, so its extra waits hoist like compute ops
                        # (semaphores are monotonic — issue-time enforcement
                        # is sound). SWDGE ring entries cannot be split.
                        raise AssertionError(
                            f"DMA {inst.name} has {len(si.on_wait)} waits; "
                            "restructure so DMAs carry at most one"
                        )
                    for w in si.on_wait[:-1]:
                        n += 1
                        out.append(
                            mybir.InstEventSemaphore(
                                name=f"I-wsplit-{n}",
                                engine=inst.engine,
                                ins=[],
                                outs=[],
                                sync_info=mybir.SyncInfo(on_wait=[w], on_update=[]),
                            )
                        )
                    inst.sync_info = mybir.SyncInfo(
                        on_wait=[si.on_wait[-1]], on_update=si.on_update
                    )
                out.append(inst)
            blk.instructions = out
    return n


def prep_inputs(
    hidden_states, cos, sin, Wq, bq, Wo, bo, Wkva, bkva, w_kc, w_vc, sinks
):
    """Build the 8 per-core input dicts (numpy, fp16/fp32)."""
    f16 = np.float16
    hs = np.asarray(hidden_states, np.float32)[0]  # [S, HID]
    cos = np.asarray(cos, np.float32)[0]  # [S, ROPE]
    sin = np.asarray(sin, np.float32)[0]

    def sbuf_pack(a):
        # [K*128, F] -> [128, K*F]: row p holds the SBUF tile [128, K, F]
        kf_, f_ = a.shape
        k_ = kf_ // 128
        return np.ascontiguousarray(
            a.reshape(k_, 128, f_).transpose(1, 0, 2).reshape(128, k_ * f_)
        )

    wqT = sbuf_pack(np.asarray(Wq, np.float32).T).astype(f16)
    wkvaT = sbuf_pack(np.asarray(Wkva, np.float32).T).astype(f16)
    wkc_p = sbuf_pack(
        np.asarray(w_kc, np.float32).transpose(2, 0, 1).reshape(R, H * NOPE)
    ).astype(f16)
    wvc_p = sbuf_pack(
        np.asarray(w_vc, np.float32).transpose(1, 0, 2).reshape(R, H * V)
    ).astype(f16)
    woT = sbuf_pack(np.asarray(Wo, np.float32).T).astype(f16)

    bq_t = np.ascontiguousarray(np.asarray(bq, np.float32).reshape(16, 128).T)
    bkva_pad = np.zeros(640, np.float32)
    bkva_pad[: R + ROPE] = np.asarray(bkva, np.float32)
    bkva_t = np.ascontiguousarray(bkva_pad.reshape(5, 128).T)
    bo_b = np.ascontiguousarray(
        np.broadcast_to(np.asarray(bo, np.float32), (128, HID))
    ).astype(f16)
    esink_b = np.ascontiguousarray(
        np.broadcast_to(np.exp(np.asarray(sinks, np.float32))[None, :], (128, H))
    )
    ident = np.eye(128, dtype=f16)

    hs_pad = np.zeros((SW + S, HID), np.float32)
    hs_pad[SW:] = hs

    shared = dict(
        wqT=wqT, wkvaT=wkvaT, wkc=wkc_p, wvc=wvc_p, woT=woT,
        bq=bq_t, bkva=bkva_t, bo=bo_b, esink=esink_b, ident=ident,
    )

    in_maps = []
    for c in range(NCORES):
        g0 = c * Q
        hsT_c = sbuf_pack(np.ascontiguousarray(hs_pad[g0 : g0 + KH].T)).astype(f16)

        cq = cos[g0 : g0 + Q]  # [Q, 64]
        sq = sin[g0 : g0 + Q]
        tqc = np.zeros((128, Q), np.float32)
        tqs = np.zeros((128, Q), np.float32)
        tqc[64:96] = cq[:, 0:32].T
        tqc[96:128] = cq[:, 32:64].T
        tqs[64:96] = sq[:, 0:32].T
        tqs[96:128] = sq[:, 32:64].T

        kpos = np.clip(np.arange(g0 - SW, g0 + Q), 0, None)
        ck = cos[kpos]  # [KH, 64]
        sk = sin[kpos]
        tkk = np.zeros((64, 2, KH), np.float32)
        tkk[0:32, 0] = ck[:, 0:32].T
        tkk[32:64, 0] = ck[:, 32:64].T
        tkk[0:32, 1] = sk[:, 0:32].T
        tkk[32:64, 1] = sk[:, 32:64].T

        jg = (g0 - SW) + np.arange(KH)  # global key index per (jb, p)
        ig = g0 + np.arange(Q)
        msk = np.zeros((128, 4, 128), np.float32)
        for s_, (it, jb) in enumerate(MASK_SLOTS):
            jj = jg[jb * 128 : (jb + 1) * 128][:, None]  # [128, 1]
            ii = ig[it * 128 : (it + 1) * 128][None, :]  # [1, 128]
            msk[:, s_, :] = (
                (jj >= 0) & (jj <= ii) & (ii - jj < SW)
            ).astype(np.float32)
        jmask = np.ascontiguousarray(
            np.broadcast_to((jg >= 0).astype(np.float32)[None, :], (128, KH))
        )
        jvalid = (jg.reshape(NJB, 128).T >= 0).astype(np.float32)  # [128, NJB]

        in_maps.append(
            dict(
                shared,
                hsT=hsT_c,
                trigq_cos=tqc.astype(f16),
                trigq_sin=tqs.astype(f16),
                trigk=tkk.astype(f16),
                mask4=msk.astype(f16),
                jmask=jmask.astype(f16),
                jvalid=jvalid.astype(f16),
            )
        )
    return in_maps


def get_program():
    if "nc" not in _CACHE:
        _CACHE["nc"] = _build_program()
    return _CACHE["nc"]


def run(in_maps, **kw):
    from concourse.bass_utils import run_bass_kernel_spmd

    nc = get_program()
    return run_bass_kernel_spmd(nc, in_maps, list(range(NCORES)), **kw)


def kernel(**inputs):
    in_maps = prep_inputs(**inputs)
    res = run(in_maps)
    out = np.concatenate([res.results[c]["out"] for c in range(NCORES)], axis=0)
    return out.reshape(B, S, HID).astype(np.float32)
